# revision 8
# baseline (speedup 1.0000x reference)
"""Trainium2 Bass kernel for nn_EquiformerV2Conv (gnn_message_passing).

Math: per-edge rotations R cancel against R^T around the per-l channel mix,
so the network reduces to
    G   = segment_sum(x[src], dst)
    ew  = mean_e sigmoid(h(d_e) + pb),  h(d) = cut(d)*sum_k pw_k gauss_k(d)
    out = [ silu(layernorm(G0 @ W0)) | (ew/sqrt(32)) * per-xyz G1 @ W1 ]
(LayerNorm is scale-invariant, so ew and the 1/sqrt(64)/E factors drop from
the l=0 path; validated numerically, rel err ~3e-3 from bf16 quantization.)

Device dataflow per core (8-way edge partition, x replicated):
  - dma_gather of 1024B pair-rows (two nodes per row, idx=src>>1 keeps int16
    range) -- ONE gather call per 3-window group; window slots split into
    even-src/odd-src halves whose matmul operands read the low/high half of
    the gathered row.  The Pool engine descriptor-gen (~7.4ns/idx) is the
    critical path; everything below hides under it.
  - one-hot S matrices are host-built and streamed from HBM (DMA is idle).
  - scatter-add = S^T @ X matmuls into per-window PSUM (51 windows x 128
    slots), transpose to channel-major, global W-mix (K=64/32 matmuls over
    384-col chunks), transpose back, LayerNorm+SiLU, store -- all emitted at
    3-window cadence so the post-gather tail is only ~12us.
  - ew: fp32 Clenshaw of a host-fitted deg-23 Chebyshev of h(d) on w=0.4d-1
    (coefficients are data-dependent inputs), Sigmoid+accumulate on Scalar,
    cross-partition sum via ones-matmul, 8-core AllReduce mid-loop.
"""
import os
import numpy as np
import ml_dtypes

bf16 = ml_dtypes.bfloat16
f32 = np.float32

# problem constants
N = 50000
E = 400000
SC, VC, DIM, NB = 64, 32, 160, 64
CUTOFF, EPS = 5.0, 1e-5

# distribution constants
P = 128            # slots per block
W = 51             # windows per core
HALF = 512         # per-window capacity per parity half
WSLOTS = 2 * HALF  # 1024 slots per window
NCORES = 8
BLKH = HALF // P            # 4 blocks per half
BLKW = 2 * BLKH             # 8 blocks per window
NBLK = W * BLKW             # 408
SLOTS = NBLK * P            # 52224
GW = 3                      # windows per gather call / pipeline chunk
NKC = W // GW               # 17
ELEM2 = 512                 # bf16 elements per pair-row (1024 B, 2 nodes)
OUTROWS = W * P             # 6528
NCOEF = 24                  # Chebyshev coefficients (deg 23)
MIXN = GW * P               # 384 mix-chunk columns
GCOLS = GW * BLKW * 8       # 192 gidx columns per gather call


# ---------------------------------------------------------------- host side

def _pack_nodes(src, dst):
    isA = (src & 1) == 0
    degA = np.bincount(dst[isA], minlength=N)
    degB = np.bincount(dst[~isA], minlength=N)
    order = np.argsort(-(degA + degB), kind="stable")
    nbins = NCORES * W
    binA = np.zeros(nbins, np.int64)
    binB = np.zeros(nbins, np.int64)
    binC = np.zeros(nbins, np.int64)
    node2win = np.full(N, -1, np.int64)
    node2slot = np.full(N, -1, np.int64)
    start = 0
    for n in order:
        a, b = degA[n], degB[n]
        for k in range(nbins):
            w = (start + k) % nbins
            if binA[w] + a <= HALF and binB[w] + b <= HALF and binC[w] < P:
                node2win[n] = w
                node2slot[n] = binC[w]
                binA[w] += a
                binB[w] += b
                binC[w] += 1
                start = (w + 1) % nbins
                break
        else:
            raise RuntimeError(f"window packing failed at node {n}")
    return node2win, node2slot


def _fit_poly(cent, wid, pw):
    """Chebyshev fit of h(d) = cut(d)*sum_k pw_k gauss_k(d) on w = 0.4d-1."""
    dg = np.linspace(0.0, CUTOFF, 6000)
    g = np.exp(-0.5 * ((dg[:, None] - cent) / wid) ** 2)
    cut = 0.5 * (np.cos(np.pi * dg / CUTOFF) + 1.0)
    h = (g @ pw) * cut
    ser = np.polynomial.chebyshev.Chebyshev.fit(0.4 * dg - 1.0, h, NCOEF - 1,
                                                domain=[-1, 1])
    return ser.coef.astype(f32)


def _stage(x, pos, src, dst):
    """Build all per-core device input arrays."""
    node2win, node2slot = _pack_nodes(src, dst)
    win_core = node2win % NCORES
    win_local = node2win // NCORES

    # pair-rows: [x[2k] 160 | pad | x[2k+1] 160 | pad], l1 cols j-major
    perm = np.arange(DIM)
    l1 = np.arange(SC, DIM)
    cc = (l1 - SC) // 3
    jj = (l1 - SC) % 3
    perm[SC + 32 * jj + cc] = l1
    xr = x[:, perm].astype(bf16)
    xp2 = np.zeros((N // 2, ELEM2), bf16)
    xp2[:, 0:DIM] = xr[0::2]
    xp2[:, ELEM2 // 2:ELEM2 // 2 + DIM] = xr[1::2]

    e_core = win_core[dst]
    e_wl = win_local[dst]
    e_slot = node2slot[dst]
    e_isB = (src & 1).astype(np.int64)

    dvec = (pos[src] - pos[dst]).astype(np.float64)
    de = np.sqrt((dvec ** 2).sum(1))
    we = (np.minimum(0.4 * de, 2.0) - 1.0).astype(f32)

    # slot assignment: group edges by (core, window, half); cumcount in group
    key = ((e_core * W + e_wl) * 2 + e_isB)
    order = np.argsort(key, kind="stable")
    ks = key[order]
    grp_start = np.searchsorted(ks, np.arange(2 * NCORES * W))
    within = np.arange(E) - grp_start[ks]
    kb = ks % (2 * W)
    wl = kb // 2
    half = kb % 2
    slot_sorted = wl * WSLOTS + half * HALF + within
    e_sorted = order

    core_sorted = e_core[e_sorted]
    ins = []
    meta_edges = []
    for r in range(NCORES):
        sel = core_sorted == r
        es = e_sorted[sel]
        sl = slot_sorted[sel]
        gidx_f = np.zeros(SLOTS, np.int16)
        dstw_f = np.full(SLOTS, -1, np.int64)
        w_f = np.ones(SLOTS, f32)
        gidx_f[sl] = (src[es] >> 1).astype(np.int16)
        dstw_f[sl] = e_slot[es]
        w_f[sl] = we[es]
        meta_edges.append(len(es))

        gidx = gidx_f.reshape(SLOTS // 16, 16).T.copy()     # [16, S/16]
        gidx = np.tile(gidx, (8, 1))                        # [128, S/16]
        wcl = w_f.reshape(NBLK, P).T.copy()                 # [128, NBLK]

        smat = np.zeros((NBLK, P, P), bf16)                 # [blk, edge_p, slot]
        blk = np.arange(SLOTS) // P
        pp = np.arange(SLOTS) % P
        valid = dstw_f >= 0
        smat[blk[valid], pp[valid], dstw_f[valid]] = bf16(1.0)
        smat = smat.transpose(1, 0, 2).reshape(P, NBLK * P)

        ins.append(dict(gidx=gidx, gidx0=gidx[:, :GCOLS].copy(),
                        wcl=wcl, smat=smat))

    meta = dict(node2win=node2win, node2slot=node2slot,
                win_core=win_core, win_local=win_local, n_edges=meta_edges)
    return xp2, ins, meta


# ---------------------------------------------------------------- device side

_PROG = None


def _build_program():
    import concourse.bacc as bacc
    import concourse.tile as tile
    from concourse import mybir, library_config

    dt = mybir.dt
    Alu = mybir.AluOpType
    Act = mybir.ActivationFunctionType

    nc = bacc.Bacc("TRN2", target_bir_lowering=False, debug=False,
                   num_devices=NCORES)

    xp2_d = nc.dram_tensor("xp2", [N // 2, ELEM2], dt.bfloat16, kind="ExternalInput")
    gidx0_d = nc.dram_tensor("gidx0", [P, GCOLS], dt.int16, kind="ExternalInput")
    gidx_d = nc.dram_tensor("gidx", [P, SLOTS // 16], dt.int16, kind="ExternalInput")
    smat_d = nc.dram_tensor("smat", [P, SLOTS], dt.bfloat16, kind="ExternalInput")
    wcl_d = nc.dram_tensor("wcl", [P, NBLK], dt.float32, kind="ExternalInput")
    coef_d = nc.dram_tensor("coef", [P, NCOEF], dt.float32, kind="ExternalInput")
    corr_d = nc.dram_tensor("corr", [1, 1], dt.float32, kind="ExternalInput")
    pbb_d = nc.dram_tensor("pbb", [P, 1], dt.float32, kind="ExternalInput")
    w0_d = nc.dram_tensor("w0", [SC, SC], dt.bfloat16, kind="ExternalInput")
    w1_d = nc.dram_tensor("w1", [VC, VC], dt.bfloat16, kind="ExternalInput")
    identb_d = nc.dram_tensor("identb", [P, P], dt.bfloat16, kind="ExternalInput")
    gamr_d = nc.dram_tensor("gamr", [P, SC], dt.float32, kind="ExternalInput")
    betr_d = nc.dram_tensor("betr", [P, SC], dt.float32, kind="ExternalInput")
    onesc_d = nc.dram_tensor("onesc", [P, 1], dt.float32, kind="ExternalInput")
    onesr_d = nc.dram_tensor("onesr", [1, P], dt.float32, kind="ExternalInput")
    out_d = nc.dram_tensor("out", [OUTROWS, DIM], dt.float32, kind="ExternalOutput")

    s1 = float(1.0 / np.sqrt(VC) / E)
    H2 = ELEM2 // 2

    with tile.TileContext(nc, num_cores=NCORES) as tc:
        import contextlib
        with contextlib.ExitStack() as ctx:
            consts = ctx.enter_context(tc.tile_pool(name="consts", bufs=1))
            gbuf = ctx.enter_context(tc.tile_pool(name="gbuf", bufs=1))
            gather = ctx.enter_context(tc.tile_pool(name="gather", bufs=2))
            spool = ctx.enter_context(tc.tile_pool(name="spool", bufs=2))
            obp = ctx.enter_context(tc.tile_pool(name="obp", bufs=2))
            gcp = ctx.enter_context(tc.tile_pool(name="gcp", bufs=2))
            mcp = ctx.enter_context(tc.tile_pool(name="mcp", bufs=4))
            ocp = ctx.enter_context(tc.tile_pool(name="ocp", bufs=8))
            zpool = ctx.enter_context(tc.tile_pool(name="zpool", bufs=3))
            lnp = ctx.enter_context(tc.tile_pool(name="lnp", bufs=4))
            psG = ctx.enter_context(tc.tile_pool(name="psG", bufs=2, space="PSUM"))
            psT = ctx.enter_context(tc.tile_pool(name="psT", bufs=2, space="PSUM"))
            psM = ctx.enter_context(tc.tile_pool(name="psM", bufs=1, space="PSUM"))
            psT2 = ctx.enter_context(tc.tile_pool(name="psT2", bufs=1, space="PSUM"))
            psE = ctx.enter_context(tc.tile_pool(name="psE", bufs=1, space="PSUM"))
            dram = ctx.enter_context(tc.tile_pool(name="dram", bufs=1, space="DRAM"))

            nc.gpsimd.load_library(library_config.mlp)

            # ---- constant loads (gidx0 first: the kc=0 gather needs only it)
            gidx0 = consts.tile([P, GCOLS], dt.int16)
            nc.sync.dma_start(out=gidx0[:], in_=gidx0_d[:])
            gidx = consts.tile([P, SLOTS // 16], dt.int16)
            nc.sync.dma_start(out=gidx[:], in_=gidx_d[:])
            wcl = consts.tile([P, NBLK], dt.float32)
            nc.sync.dma_start(out=wcl[:], in_=wcl_d[:])
            coef = consts.tile([P, NCOEF], dt.float32)
            nc.sync.dma_start(out=coef[:], in_=coef_d[:])
            corrt = consts.tile([1, 1], dt.float32)
            nc.sync.dma_start(out=corrt[:], in_=corr_d[:])
            pbb = consts.tile([P, 1], dt.float32)
            nc.sync.dma_start(out=pbb[:], in_=pbb_d[:])
            w0sb = consts.tile([SC, SC], dt.bfloat16)
            nc.sync.dma_start(out=w0sb[:], in_=w0_d[:])
            w1sb = consts.tile([VC, VC], dt.bfloat16)
            nc.sync.dma_start(out=w1sb[:], in_=w1_d[:])
            identb = consts.tile([P, P], dt.bfloat16)
            nc.sync.dma_start(out=identb[:], in_=identb_d[:])
            gamr = consts.tile([P, SC], dt.float32)
            nc.sync.dma_start(out=gamr[:], in_=gamr_d[:])
            betr = consts.tile([P, SC], dt.float32)
            nc.sync.dma_start(out=betr[:], in_=betr_d[:])
            onesc = consts.tile([P, 1], dt.float32)
            nc.sync.dma_start(out=onesc[:], in_=onesc_d[:])
            onesr = consts.tile([1, P], dt.float32)
            nc.sync.dma_start(out=onesr[:], in_=onesr_d[:])

            # ---- z-phase: Clenshaw of deg-23 Chebyshev, sigmoid+accumulate
            w2c = gbuf.tile([P, NBLK], dt.float32)
            nc.vector.tensor_scalar(out=w2c[:], in0=wcl[:], scalar1=2.0,
                                    scalar2=None, op0=Alu.mult)
            b1 = zpool.tile([P, NBLK], dt.float32, tag="zb")
            nc.vector.tensor_scalar(out=b1[:], in0=wcl[:], scalar1=0.0,
                                    scalar2=coef[:, NCOEF - 1:NCOEF],
                                    op0=Alu.mult, op1=Alu.add)
            b2 = zpool.tile([P, NBLK], dt.float32, tag="zb")
            nc.vector.memset(b2[:], 0.0)
            for k in range(NCOEF - 2, 0, -1):
                t = zpool.tile([P, NBLK], dt.float32, tag="zt")
                nc.vector.tensor_tensor(out=t[:], in0=w2c[:], in1=b1[:],
                                        op=Alu.mult)
                bn = zpool.tile([P, NBLK], dt.float32, tag="zb")
                nc.vector.scalar_tensor_tensor(
                    out=bn[:], in0=t[:], scalar=coef[:, k:k + 1], in1=b2[:],
                    op0=Alu.add, op1=Alu.subtract)
                b2, b1 = b1, bn
            tf = zpool.tile([P, NBLK], dt.float32, tag="zt")
            nc.vector.tensor_tensor(out=tf[:], in0=wcl[:], in1=b1[:], op=Alu.mult)
            uz = gbuf.tile([P, NBLK], dt.float32)
            nc.vector.scalar_tensor_tensor(
                out=uz[:], in0=tf[:], scalar=coef[:, 0:1], in1=b2[:],
                op0=Alu.add, op1=Alu.subtract)
            zscr = gbuf.tile([P, NBLK], dt.float32)
            zsum = gbuf.tile([P, 1], dt.float32)
            nc.scalar.activation(out=zscr[:], in_=uz[:], func=Act.Sigmoid,
                                 bias=pbb[:, 0:1], accum_out=zsum[:, 0:1])

            ewc1 = gbuf.tile([P, 1], dt.float32)
            outv = out_d[:].rearrange("(w p) d -> p w d", p=P)
            pending = []

            for kc in range(NKC):
                sload = spool.tile([P, GW * BLKW, P], dt.bfloat16, tag="sload")
                nc.sync.dma_start(
                    out=sload[:],
                    in_=smat_d[:, kc * GW * BLKW * P:(kc + 1) * GW * BLKW * P])
                xg = gather.tile([P, GW * BLKW, ELEM2], dt.bfloat16, tag="xg")
                idxs = gidx0[:] if kc == 0 else gidx[:, kc * GCOLS:(kc + 1) * GCOLS]
                nc.gpsimd.dma_gather(
                    xg[:], xp2_d[:, :], idxs, GW * BLKW * P, GW * BLKW * P,
                    ELEM2, single_packet=False)

                obw = obp.tile([P, GW, DIM], dt.bfloat16, tag="obw")
                gpk = gcp.tile([SC, 1 + 3, MIXN], dt.bfloat16, tag="gpk")
                for wi in range(GW):
                    gps = psG.tile([P, DIM], dt.float32, tag="gps")
                    for b in range(BLKW):
                        sl = slice(0, DIM) if b < BLKH else slice(H2, H2 + DIM)
                        nc.tensor.matmul(
                            gps[:], sload[:, wi * BLKW + b, :],
                            xg[:, wi * BLKW + b, sl],
                            start=(b == 0), stop=(b == BLKW - 1))
                    nc.scalar.copy(out=obw[:, wi, :], in_=gps[:])
                    # transpose to channel-major; all outputs partition-base 0
                    tpk = psT.tile([SC, 4, P], dt.bfloat16, tag="tpk")
                    nc.tensor.transpose(out=tpk[:, 0, :], in_=obw[:, wi, 0:SC],
                                        identity=identb[:])
                    for j in range(3):
                        nc.tensor.transpose(
                            out=tpk[0:VC, 1 + j, :],
                            in_=obw[:, wi, SC + VC * j:SC + VC * (j + 1)],
                            identity=identb[:])
                    nc.scalar.copy(out=gpk[:, :, wi * P:(wi + 1) * P], in_=tpk[:])

                # ---- mix chunk kc (3 windows, channel-major)
                mp = psM.tile([SC, MIXN], dt.float32, tag="mp")
                nc.tensor.matmul(mp[:], w0sb[:], gpk[:, 0, :], start=True, stop=True)
                m0c = mcp.tile([SC, MIXN], dt.bfloat16, tag="m0c")
                nc.vector.tensor_copy(out=m0c[:], in_=mp[:])
                m1c = []
                for j in range(3):
                    m1p = psM.tile([VC, MIXN], dt.float32, tag="m1p")
                    nc.tensor.matmul(m1p[:], w1sb[:], gpk[0:VC, 1 + j, :],
                                     start=True, stop=True)
                    m1t = mcp.tile([VC, MIXN], dt.bfloat16, tag=f"m1c{j}",
                                   name=f"m1c{j}")
                    nc.vector.tensor_copy(out=m1t[:], in_=m1p[:])
                    m1c.append(m1t)

                # ---- transpose back to node-major
                och = ocp.tile([P, GW, DIM], dt.float32, tag="och")
                for wi in range(GW):
                    lo = wi * P
                    t2pk = psT2.tile([P, DIM], dt.bfloat16, tag="t2pk")
                    nc.tensor.transpose(out=t2pk[:, 0:SC], in_=m0c[:, lo:lo + P],
                                        identity=identb[0:SC, 0:SC])
                    for j in range(3):
                        nc.tensor.transpose(
                            out=t2pk[:, SC + VC * j:SC + VC * (j + 1)],
                            in_=m1c[j][:, lo:lo + P],
                            identity=identb[0:VC, 0:VC])
                    nc.scalar.copy(out=och[:, wi, :], in_=t2pk[:])

                if kc == 1:
                    # cross-partition sum of zsum via ones-matmul (Pool stays free)
                    ewt = psE.tile([P, 2], dt.float32, tag="ewt", name="ewt")
                    nc.tensor.matmul(ewt[0:1, 0:1], onesc[:], zsum[:],
                                     start=True, stop=True)
                    zc8 = gbuf.tile([1, 8], dt.float32)
                    nc.vector.tensor_scalar(
                        out=zc8[:], in0=ewt[0:1, 0:1].to_broadcast([1, 8]),
                        scalar1=corrt[0:1, 0:1], scalar2=None, op0=Alu.subtract)
                    arin = dram.tile([1, 8], dt.float32)
                    arout = dram.tile([1, 8], dt.float32)
                    nc.sync.dma_start(out=arin[:], in_=zc8[:])
                    nc.gpsimd.collective_compute(
                        "AllReduce", Alu.add, replica_groups=[list(range(NCORES))],
                        ins=[arin.opt()], outs=[arout.opt()])
                if kc == 5:
                    ewsb = gbuf.tile([1, 8], dt.float32)
                    nc.sync.dma_start(out=ewsb[:], in_=arout[:])
                    ewg = gbuf.tile([1, 1], dt.float32)
                    nc.vector.tensor_scalar(out=ewg[:], in0=ewsb[0:1, 0:1],
                                            scalar1=s1, scalar2=None, op0=Alu.mult)
                    nc.tensor.matmul(ewt[:, 1:2], onesr[:], ewg[:],
                                     start=True, stop=True)
                    nc.scalar.copy(out=ewc1[:], in_=ewt[:, 1:2])

                # ---- layernorm + silu on l0, ew-scale l1, store
                ob0 = och[:, :, 0:SC]
                mu = lnp.tile([P, GW], dt.float32, tag="mu")
                nc.vector.tensor_reduce(out=mu[:], in_=ob0,
                                        axis=mybir.AxisListType.X, op=Alu.add)
                mu2 = lnp.tile([P, GW], dt.float32, tag="mu2")
                nc.vector.tensor_scalar(out=mu2[:], in0=mu[:],
                                        scalar1=float(1.0 / SC), scalar2=None,
                                        op0=Alu.mult)
                cen = lnp.tile([P, GW, SC], dt.float32, tag="cen")
                nc.vector.tensor_tensor(
                    out=cen[:], in0=ob0,
                    in1=mu2[:].unsqueeze(2).to_broadcast([P, GW, SC]),
                    op=Alu.subtract)
                sq = lnp.tile([P, GW, SC], dt.float32, tag="lnt")
                nc.vector.tensor_tensor(out=sq[:], in0=cen[:], in1=cen[:],
                                        op=Alu.mult)
                varb = lnp.tile([P, GW], dt.float32, tag="mu")
                nc.vector.tensor_reduce(out=varb[:], in_=sq[:],
                                        axis=mybir.AxisListType.X, op=Alu.add)
                vb2 = lnp.tile([P, GW], dt.float32, tag="mu2")
                nc.vector.tensor_scalar(out=vb2[:], in0=varb[:],
                                        scalar1=float(1.0 / SC), scalar2=float(EPS),
                                        op0=Alu.mult, op1=Alu.add)
                sdb = lnp.tile([P, GW], dt.float32, tag="mu")
                nc.scalar.activation(out=sdb[:], in_=vb2[:], func=Act.Sqrt)
                rsb = lnp.tile([P, GW], dt.float32, tag="mu2")
                nc.vector.reciprocal(out=rsb[:], in_=sdb[:])
                t1b_ = lnp.tile([P, GW, SC], dt.float32, tag="lnt")
                nc.vector.tensor_tensor(
                    out=t1b_[:], in0=cen[:],
                    in1=rsb[:].unsqueeze(2).to_broadcast([P, GW, SC]),
                    op=Alu.mult)
                t2b_ = lnp.tile([P, GW, SC], dt.float32, tag="lnt")
                nc.vector.tensor_tensor(
                    out=t2b_[:], in0=t1b_[:],
                    in1=gamr[:].unsqueeze(1).to_broadcast([P, GW, SC]),
                    op=Alu.mult)
                t3b_ = lnp.tile([P, GW, SC], dt.float32, tag="lnt")
                nc.vector.tensor_tensor(
                    out=t3b_[:], in0=t2b_[:],
                    in1=betr[:].unsqueeze(1).to_broadcast([P, GW, SC]),
                    op=Alu.add)
                sgb = lnp.tile([P, GW, SC], dt.float32, tag="lnt")
                nc.scalar.activation(out=sgb[:], in_=t3b_[:], func=Act.Sigmoid)
                nc.vector.tensor_tensor(out=ob0, in0=t3b_[:], in1=sgb[:],
                                        op=Alu.mult)
                if kc < 5:
                    # ewc1 (AllReduce result) is not written until kc==5;
                    # defer the l1 scale + store (read-before-write would
                    # race in the tile framework's program-order deps)
                    pending.append((kc, och))
                else:
                    for pkc, poch in pending:
                        nc.vector.tensor_scalar(
                            out=poch[:, :, SC:DIM], in0=poch[:, :, SC:DIM],
                            scalar1=ewc1[:, 0:1], scalar2=None, op0=Alu.mult)
                        nc.scalar.dma_start(
                            out=outv[:, pkc * GW:(pkc + 1) * GW, :], in_=poch[:])
                    pending.clear()
                    nc.vector.tensor_scalar(
                        out=och[:, :, SC:DIM], in0=och[:, :, SC:DIM],
                        scalar1=ewc1[:, 0:1], scalar2=None, op0=Alu.mult)
                    nc.scalar.dma_start(out=outv[:, kc * GW:(kc + 1) * GW, :],
                                        in_=och[:])


    nc.compile()
    return nc


def _get_program():
    global _PROG
    if _PROG is None:
        _PROG = _build_program()
    return _PROG


# ---------------------------------------------------------------- entry point

def kernel(**inputs):
    from concourse.bass_utils import run_bass_kernel_spmd

    x = np.asarray(inputs["x"], f32)
    pos = np.asarray(inputs["pos"], f32)
    ei = np.asarray(inputs["edge_index"])
    src = ei[0].astype(np.int64)
    dst = ei[1].astype(np.int64)

    xp2, cores, meta = _stage(x, pos, src, dst)

    cent = np.asarray(inputs["rbf_centers"], np.float64).reshape(-1)
    wid = np.asarray(inputs["rbf_widths"], np.float64).reshape(-1)
    pw = np.asarray(inputs["edge_proj_w"], np.float64).reshape(-1)
    pb = float(np.asarray(inputs["edge_proj_b"]).reshape(-1)[0])
    coefs = _fit_poly(cent, wid, pw)
    sig_pb = 1.0 / (1.0 + np.exp(-pb))

    common = dict(
        xp2=xp2,
        coef=np.tile(coefs[None, :], (P, 1)).astype(f32),
        pbb=np.full((P, 1), pb, f32),
        w0=np.asarray(inputs["W0"], f32).astype(bf16),
        w1=np.asarray(inputs["W1"], f32).astype(bf16),
        identb=np.eye(P, dtype=bf16),
        gamr=np.tile(np.asarray(inputs["ln_gamma"], f32).reshape(1, SC), (P, 1)),
        betr=np.tile(np.asarray(inputs["ln_beta"], f32).reshape(1, SC), (P, 1)),
        onesc=np.ones((P, 1), f32),
        onesr=np.ones((1, P), f32),
    )
    in_maps = []
    for r in range(NCORES):
        cd = cores[r]
        n_empty = SLOTS - meta["n_edges"][r]
        in_maps.append(dict(
            common, gidx=cd["gidx"], gidx0=cd["gidx0"], wcl=cd["wcl"],
            smat=cd["smat"], corr=np.array([[n_empty * sig_pb]], f32)))

    nc = _get_program()
    trace = bool(int(os.environ.get("KERNEL_TRACE", "0")))
    res = run_bass_kernel_spmd(nc, in_maps, core_ids=list(range(NCORES)),
                               trace=trace)
    kernel.last_results = res

    # assemble full output
    out_full = np.zeros((N, DIM), f32)
    col_map = np.arange(DIM)
    for jj in range(3):
        for cc in range(VC):
            col_map[SC + 3 * cc + jj] = SC + VC * jj + cc
    n2w, n2s = meta["node2win"], meta["node2slot"]
    wc, wl = meta["win_core"], meta["win_local"]
    for r in range(NCORES):
        o = res.results[r]["out"]                      # [W*P, DIM]
        nodes = np.nonzero(wc == r)[0]
        rows = wl[nodes] * P + n2s[nodes]
        out_full[nodes] = o[rows][:, col_map]
    return out_full
